# revision 53
# baseline (speedup 1.0000x reference)
"""Multi-head attention (B=2, S=4096, E=768, H=12, D=64) on 8 TRN2 NeuronCores.

Sharding: data parallel over batch (2) x tensor parallel over head groups (4):
core c handles batch c//4, heads 3*(c%4) .. 3*(c%4)+2.

Per-core kernel (fp16 matmul inputs, fp32 accumulation), structured to minimize
TimelineSim cost (matmul cost = output free size; LDWEIGHTS free):

  phase 1: K^T (a-scaled), {Q2|a*K2}, and V projections from x^T (x arrives per
    s-block so matmuls start ~4us in). The a = 184.665 Schraudolph scale is
    folded into the K weights on the host so score psums arrive pre-scaled for
    both exp paths. The first q-block's head-2 scores+exp are folded into this
    phase (their inputs become ready per s-block), so the attention pipeline
    starts saturated.
  phase 2: head-phase pipeline over 24 (q-block, head) blocks. During block
    p's 32 score matmuls [128k x 512q] + exp steps, block p-1's attn@V runs.
    exp alternates engines per k-tile: exact exp on ScalarE (scale = SCALE/a),
    Schraudolph exp2 on VectorE (bits = int16(s + 15301) bitcast to fp16;
    ~1.8% rms on half the weights, mean calibrated out; final l2 ~6e-3) into
    a per-block es buffer [128, 32kt, 512] (double buffered).
    attn@V is es-stationary: out [128q, 65] per matmul (free size 65, half
    the cost of the V-stationary form); col 64 of V carries ones so the
    softmax denominator falls out of the accumulation. The four per-q-chunk
    accumulation chains are each emitted contiguously (PSUM chains within one
    bank must not interleave; cross-bank interleave with scores is fine),
    4 MMs per step. Normalize = reciprocal + per-partition tensor_scalar
    (split ACT/DVE), PE-transpose [q,64]->[64,q] (deferred a few steps so the
    PE never waits on the normalize), assemble attT, then the row-parallel
    projection y^T = Wp^T @ attT spread one f-tile per 2 steps, DMA per
    (f-tile, qb) straight out of SBUF staging.
Host: y[b] = sum of the 4 partial y^T.T per batch + b_proj.
"""
import numpy as np

EMBED = 768
SEQ = 4096
NHEAD_CORE = 3          # heads per core
DHEAD = 64
DSL = NHEAD_CORE * DHEAD  # 192: per-core head-dim slice
QB = 512                # q-block
NQB = SEQ // QB         # 8
NKT = SEQ // 128        # 32 k-tiles
NPAIR = NKT // 2        # 16 k-tile pairs per head
NEC = EMBED // 128      # 6 e-chunks
SCALE = DHEAD ** -0.5
A_FOLD = 184.665        # 1024*log2(e)*SCALE, folded into K weights on host
ACT_SCALE = SCALE / A_FOLD
SCH_BIAS = 15301.0      # fp16-bits exp2 bias, mean-error calibrated on device

_CACHED = {}
_ALL_ACT = False


def _build():
    import concourse.bacc as bacc
    import concourse.tile as tile
    from concourse import mybir

    F32 = mybir.dt.float32
    F16 = mybir.dt.float16
    I16 = mybir.dt.int16
    EXP = mybir.ActivationFunctionType.Exp
    ADD = mybir.AluOpType.add
    MULT = mybir.AluOpType.mult

    nc = bacc.Bacc("TRN2")
    xT_d = nc.dram_tensor("xT", [EMBED, SEQ], F16, kind="ExternalInput")
    wq_d = nc.dram_tensor("wq", [EMBED, 128], F16, kind="ExternalInput")
    wk_d = nc.dram_tensor("wk", [EMBED, 128], F16, kind="ExternalInput")
    wqk2_d = nc.dram_tensor("wqk2", [EMBED, 128], F16, kind="ExternalInput")
    wv_d = nc.dram_tensor("wv", [EMBED, DSL], F16, kind="ExternalInput")
    wp_d = nc.dram_tensor("wp", [DSL, EMBED], F16, kind="ExternalInput")
    id_d = nc.dram_tensor("ident", [128, 128], F16, kind="ExternalInput")
    yT_d = nc.dram_tensor("yT", [EMBED, SEQ], F16, kind="ExternalOutput")

    with tile.TileContext(nc) as tc:
        with (
            tc.tile_pool(name="persist", bufs=1) as persist,
            tc.tile_pool(name="qtp", bufs=3) as qtp,
            tc.tile_pool(name="esp", bufs=2) as esp,
            tc.tile_pool(name="attqp", bufs=3) as attqp,
            tc.tile_pool(name="attTp", bufs=2) as attTp,
            tc.tile_pool(name="recp", bufs=3) as recp,
            tc.tile_pool(name="ysbp", bufs=4) as ysbp,
            tc.tile_pool(name="psS", bufs=4, space="PSUM") as psS,
            tc.tile_pool(name="psAV", bufs=2, space="PSUM") as psAV,
            tc.tile_pool(name="psP", bufs=2, space="PSUM") as psP,
        ):
            # ---- persistent SBUF ----
            x_sb = persist.tile([128, NEC, SEQ], F16, name="x_sb")
            wq_sb = persist.tile([128, NEC, 128], F16, name="wq_sb")
            wk_sb = persist.tile([128, NEC, 128], F16, name="wk_sb")
            wqk2_sb = persist.tile([128, NEC, 128], F16, name="wqk2_sb")
            wv_sb = persist.tile([128, NEC, DSL], F16, name="wv_sb")
            wp_a = persist.tile([128, EMBED], F16, name="wp_a")
            wp_b = persist.tile([64, EMBED], F16, name="wp_b")
            id_sb = persist.tile([128, 128], F16, name="id_sb")
            # K^T for heads 0,1 (a-scaled), packed at partition halves
            kt01 = persist.tile([128, SEQ], F16, name="kt01")
            # head 2: Q2 at [:,0,:], a*K2 at [:,1,:] -- same partition base
            qk2s = persist.tile([64, 2, SEQ], F16, name="qk2s")
            # V natural layout + ones column: [k-part, kt, head, 65]
            v_sb = persist.tile([128, NKT, NHEAD_CORE, 65], F16, name="v_sb")

            # DMAs ordered by first consumption: K weights + x block 0 first
            nc.sync.dma_start(out=wk_sb[:],
                              in_=wk_d.rearrange("(c p) d -> p c d", p=128))

            def dma_x_chunk(c):
                cc = slice(128 * c, 128 * (c + 1))
                nc.sync.dma_start(
                    out=x_sb[:, :, cc],
                    in_=xT_d[:, cc].rearrange("(c p) s -> p c s", p=128))

            dma_x_chunk(0)
            nc.sync.dma_start(out=wqk2_sb[:],
                              in_=wqk2_d.rearrange("(c p) d -> p c d", p=128))
            dma_x_chunk(1)
            nc.sync.dma_start(out=wv_sb[:],
                              in_=wv_d.rearrange("(c p) d -> p c d", p=128))
            dma_x_chunk(2)
            dma_x_chunk(3)
            for sb in range(1, NQB):
                cols = slice(QB * sb, QB * (sb + 1))
                nc.sync.dma_start(
                    out=x_sb[:, :, cols],
                    in_=xT_d[:, cols].rearrange("(c p) s -> p c s", p=128))
            nc.sync.dma_start(out=wq_sb[:],
                              in_=wq_d.rearrange("(c p) d -> p c d", p=128))
            nc.sync.dma_start(out=wp_a[:], in_=wp_d[0:128, :])
            nc.sync.dma_start(out=wp_b[:], in_=wp_d[128:DSL, :])
            nc.sync.dma_start(out=id_sb[:], in_=id_d[:, :])
            nc.vector.memset(v_sb[:, :, :, 64:65], 1.0)

            # ---- phase 1: K/Q2K2/V projections (psums from psP/psAV) ----
            def phase1_sb(sb, post_qk2=None):
                cols = slice(QB * sb, QB * (sb + 1))
                nsub = 4 if sb == 0 else 1
                kps = psP.tile([128, QB], F32, name="kps", tag="psP")
                qps = psP.tile([128, QB], F32, name="qps", tag="psP")
                for c in range(nsub):
                    sc = slice(QB * sb + 512 // nsub * c,
                               QB * sb + 512 // nsub * (c + 1))
                    oc = slice(512 // nsub * c, 512 // nsub * (c + 1))
                    for e in range(NEC):
                        nc.tensor.matmul(kps[:, oc], wk_sb[:, e, :],
                                         x_sb[:, e, sc],
                                         start=(e == 0), stop=(e == NEC - 1))
                    for e in range(NEC):
                        nc.tensor.matmul(qps[:, oc], wqk2_sb[:, e, :],
                                         x_sb[:, e, sc],
                                         start=(e == 0), stop=(e == NEC - 1))
                    if nsub == 4:
                        kt_abs = 4 * sb + c
                        scs = slice(128 * kt_abs, 128 * (kt_abs + 1))
                        vps = psAV.tile([128, DSL], F32, name="vps",
                                        tag="psAV")
                        for e in range(NEC):
                            nc.tensor.matmul(vps[:], x_sb[:, e, scs],
                                             wv_sb[:, e, :],
                                             start=(e == 0),
                                             stop=(e == NEC - 1))
                        nc.scalar.copy(
                            v_sb[:, kt_abs, :, 0:64],
                            vps[:].rearrange("p (h d) -> p h d",
                                             h=NHEAD_CORE))
                nc.scalar.copy(kt01[:, cols], kps[:])
                nc.scalar.copy(qk2s[:, 0, cols], qps[0:64, :])
                nc.scalar.copy(qk2s[:, 1, cols], qps[64:128, :])
                if post_qk2 is not None:
                    post_qk2(sb)
                if nsub == 1:
                    for c in range(4):  # V s-chunks of 128
                        kt_abs = 4 * sb + c
                        scs = slice(128 * kt_abs, 128 * (kt_abs + 1))
                        vps = psAV.tile([128, DSL], F32, name="vps",
                                        tag="psAV")
                        for e in range(NEC):
                            nc.tensor.matmul(vps[:], x_sb[:, e, scs],
                                             wv_sb[:, e, :],
                                             start=(e == 0),
                                             stop=(e == NEC - 1))
                        nc.scalar.copy(
                            v_sb[:, kt_abs, :, 0:64],
                            vps[:].rearrange("p (h d) -> p h d",
                                             h=NHEAD_CORE))

            # ---- phase 2: attention + projection ----
            # Head-phase pipeline: during head-block p's 32 score+exp steps,
            # head-block p-1's attn@V runs as four per-q-chunk accumulation
            # chains. Each chain's 32 matmuls are emitted contiguously (PSUM
            # accumulation chains within one bank must not interleave with
            # other chains in that bank; cross-bank interleave is fine), 16
            # per step over steps 0..7. exp writes a per-head es buffer
            # [128, 32, 512] (double buffered) so attn@V reads a completed
            # buffer with a full phase of slack.
            HS = [2, 0, 1]
            NPH = NQB * NHEAD_CORE      # 24 head-blocks
            DEFER = 6
            PDEFER = 4

            qt_tiles = {}
            attT_tiles = {}
            _dr = [None]
            av_tiles = {}
            es_bufs = {}
            pend = {}

            def blk(p):
                return p // NHEAD_CORE, HS[p % NHEAD_CORE]

            def emit_qproj(qb):
                qcols = slice(QB * qb, QB * (qb + 1))
                qps = psP.tile([128, QB], F32, name="qps2", tag="psP")
                for e in range(NEC):
                    nc.tensor.matmul(qps[:], wq_sb[:, e, :],
                                     x_sb[:, e, qcols],
                                     start=(e == 0), stop=(e == NEC - 1))
                qt = qtp.tile([128, QB], F16, name="qt", tag="qt")
                nc.scalar.copy(qt[:], qps[:])
                qt_tiles[qb] = qt

            def emit_scores_exp(p, kt):
                qb, h = blk(p)
                qcols = slice(QB * qb, QB * (qb + 1))
                if kt == 0:
                    es_bufs[p] = esp.tile([128, NKT, QB], F16, name="esb",
                                          tag="es")
                sps = psS.tile([128, QB], F32, name="sps", tag="psS")
                kk = slice(128 * kt, 128 * (kt + 1))
                if h < 2:
                    hp = slice(64 * h, 64 * (h + 1))
                    nc.tensor.matmul(sps[:], kt01[hp, kk],
                                     qt_tiles[qb][hp, :],
                                     start=True, stop=True)
                else:
                    nc.tensor.matmul(sps[:], qk2s[:, 1, kk],
                                     qk2s[:, 0, qcols],
                                     start=True, stop=True)
                dst = es_bufs[p][:, kt, :]
                if kt % 2 == 0 and not _ALL_ACT:
                    nc.vector.tensor_scalar(
                        out=dst.bitcast(I16), in0=sps[:],
                        scalar1=SCH_BIAS, scalar2=None, op0=ADD)
                else:
                    nc.scalar.activation(out=dst, in_=sps[:],
                                         func=EXP, scale=ACT_SCALE)

            # AV chain schedules: SCHED[k] = [(qc, kt), ...] per step.
            # Chains stay contiguous per qc; spread over 28 steps so the es
            # buffer frees early, or bunched over 8 steps for the drain phase.
            def _mk_sched(bounds):
                sched = [[] for _ in range(NKT)]
                nsteps = len(bounds) - 1
                for qc in range(4):
                    for j in range(nsteps):
                        for kt in range(bounds[j], bounds[j + 1]):
                            sched[nsteps * qc + j].append((qc, kt))
                return sched

            SCHED_MAIN = _mk_sched([0, 5, 10, 15, 20, 24, 28, 32])
            SCHED_DRAIN = _mk_sched([0, 8, 16, 24, 32])

            def emit_av(p, k, sched):
                if not sched[k]:
                    return
                qb, h = blk(p)
                if k == 0:
                    av_tiles[p] = psAV.tile([128, 4, 65], F32,
                                            name="av", tag="psAV")
                av = av_tiles[p]
                esb = es_bufs[p]
                for qc, kt in sched[k]:
                    nc.tensor.matmul(
                        av[:, qc, :], esb[:, kt, 128 * qc:128 * (qc + 1)],
                        v_sb[:, kt, h, :],
                        start=(kt == 0), stop=(kt == NKT - 1),
                        skip_group_check=True)

            def emit_norm(p):
                qb, h = blk(p)
                av = av_tiles.pop(p)
                del es_bufs[p]
                rec = recp.tile([128, 4], F32, name="rec", tag="rec")
                nc.vector.reciprocal(out=rec[:, :], in_=av[:, :, 64])
                attq = attqp.tile([128, 4, DHEAD], F16, name="attq",
                                  tag="attq")
                for qc in range(4):
                    if qc % 2 == 0:
                        nc.scalar.mul(attq[:, qc, :], av[:, qc, 0:64],
                                      rec[:, qc:qc + 1])
                    else:
                        nc.vector.tensor_scalar(
                            out=attq[:, qc, :], in0=av[:, qc, 0:64],
                            scalar1=rec[:, qc:qc + 1], scalar2=None, op0=MULT)
                return attq

            def emit_transposes(p, attq):
                qb, h = blk(p)
                if h == HS[0]:
                    attT_tiles[qb] = (
                        attTp.tile([128, QB], F16, name="attT01", tag="a01"),
                        attTp.tile([64, QB], F16, name="attT2", tag="a2"))
                attT01, attT2 = attT_tiles[qb]
                tp = psP.tile([64, 4, 128], F16, name="tp", tag="psP")
                for qc in range(4):
                    nc.tensor.transpose(tp[:, qc, :], attq[:, qc, :],
                                        id_sb[:])
                if h == 0:
                    dst = attT01[0:64, :]
                elif h == 1:
                    dst = attT01[64:128, :]
                else:
                    dst = attT2[:, :]
                nc.scalar.copy(dst.rearrange("p (c q) -> p c q", c=4), tp[:])

            def emit_proj_f(qb, f, drain=False):
                qcols = slice(QB * qb, QB * (qb + 1))
                attT01, attT2 = attT_tiles[qb]
                fc = slice(128 * f, 128 * (f + 1))
                if drain:
                    yps = psS.tile([128, QB], F32, name="yps", tag="psS")
                else:
                    yps = psP.tile([128, QB], F32, name="yps", tag="psP")
                nc.tensor.matmul(yps[:], wp_a[:, fc], attT01[:],
                                 start=True, stop=False)
                nc.tensor.matmul(yps[:], wp_b[:, fc], attT2[:],
                                 start=False, stop=True)
                if drain:
                    if f % 2 == 0:
                        _dr[0] = ysbp.tile([128, 2, QB], F16, name="ysb2",
                                           tag="ysb2")
                    ysb2 = _dr[0]
                    if f % 2 == 0:
                        nc.scalar.copy(ysb2[:, 0, :], yps[:])
                    else:
                        nc.vector.tensor_copy(ysb2[:, 1, :], yps[:])
                        fc2 = slice(128 * (f - 1), 128 * (f + 1))
                        nc.sync.dma_start(
                            out=yT_d[fc2, qcols].rearrange(
                                "(c p) q -> p c q", p=128),
                            in_=ysb2[:])
                else:
                    ysb = ysbp.tile([128, QB], F16, name="ysb", tag="ysb")
                    if f % 2 == 0:
                        nc.scalar.copy(ysb[:], yps[:])
                    else:
                        nc.vector.tensor_copy(ysb[:], yps[:])
                    nc.sync.dma_start(out=yT_d[fc, qcols], in_=ysb[:])
                if f == NEC - 1:
                    attT_tiles.pop(qb)

            def p0_scores(sb):
                for kt in range(4 * sb, 4 * sb + 4):
                    emit_scores_exp(0, kt)

            for sb in range(NQB):
                phase1_sb(sb, post_qk2=p0_scores)
            emit_qproj(0)
            for g in range(NKT, (NPH + 1) * NKT):
                p, k = divmod(g, NKT)
                if p < NPH:
                    emit_scores_exp(p, k)
                    if False:
                        pass  # qproj for next qb emitted below at own time
                    if p % NHEAD_CORE == 0 and k == 18:
                        nqb = p // NHEAD_CORE
                        if nqb not in qt_tiles:
                            emit_qproj(nqb)
                if p >= 1:
                    if p == NPH:
                        if k < 16:
                            emit_av(p - 1, k, SCHED_DRAIN)
                        if k == 16:
                            pend[g + 1] = ('tp', p - 1, emit_norm(p - 1))
                    else:
                        emit_av(p - 1, k, SCHED_MAIN)
                        if k == 28:
                            pend[g + DEFER] = ('tp', p - 1,
                                               emit_norm(p - 1))
                if g in pend:
                    item = pend.pop(g)
                    if item[0] == 'tp':
                        _, pp, attq = item
                        emit_transposes(pp, attq)
                        qb, h = blk(pp)
                        if h == HS[-1]:
                            for f in range(NEC):
                                pend[g + PDEFER + 2 * f] = ('proj', qb, f)
                    else:
                        emit_proj_f(item[1], item[2], drain=(item[1] == NQB - 1))
            for gg in sorted(pend):
                item = pend[gg]
                if item[0] == 'tp':
                    _, pp, attq = item
                    emit_transposes(pp, attq)
                    qb, h = blk(pp)
                    if h == HS[-1]:
                        for f in range(NEC):
                            emit_proj_f(qb, f)
                else:
                    emit_proj_f(item[1], item[2], drain=True)

    nc.compile()
    return nc


def _get_nc():
    if "nc" not in _CACHED:
        _CACHED["nc"] = _build()
    return _CACHED["nc"]


def _make_in_maps(x, W_qkv, W_proj):
    f16 = np.float16
    ident = np.eye(128, dtype=f16)
    in_maps = []
    for c in range(8):
        b = c // 4
        g = c % 4
        sl = slice(DSL * g, DSL * (g + 1))
        xT = np.ascontiguousarray(x[b].T).astype(f16)
        wqT = np.ascontiguousarray(W_qkv[0:EMBED][sl, :].T)          # [768,192]
        wkT = np.ascontiguousarray(W_qkv[EMBED:2 * EMBED][sl, :].T) * A_FOLD
        wvT = np.ascontiguousarray(W_qkv[2 * EMBED:3 * EMBED][sl, :].T)
        wp = np.ascontiguousarray(W_proj[:, sl].T)                   # [192,768]
        wqk2 = np.concatenate([wqT[:, 128:192], wkT[:, 128:192]], axis=1)
        in_maps.append({
            "xT": xT,
            "wq": wqT[:, 0:128].astype(f16),
            "wk": wkT[:, 0:128].astype(f16),
            "wqk2": np.ascontiguousarray(wqk2).astype(f16),
            "wv": wvT.astype(f16),
            "wp": wp.astype(f16),
            "ident": ident,
        })
    return in_maps


def kernel(x, W_qkv, W_proj, b_proj):
    from concourse.bass_utils import run_bass_kernel_spmd

    x = np.asarray(x, dtype=np.float32)
    W_qkv = np.asarray(W_qkv, dtype=np.float32)
    W_proj = np.asarray(W_proj, dtype=np.float32)
    b_proj = np.asarray(b_proj, dtype=np.float32)

    nc = _get_nc()
    in_maps = _make_in_maps(x, W_qkv, W_proj)
    res = run_bass_kernel_spmd(nc, in_maps, core_ids=list(range(8)))

    y = np.zeros((2, SEQ, EMBED), dtype=np.float32)
    for c in range(8):
        y[c // 4] += res.results[c]["yT"].astype(np.float32).T
    y += b_proj
    return y
